# revision 19
# baseline (speedup 1.0000x reference)
"""CrossSetNorm Trainium2 kernel (8 NeuronCores, batch-parallel).

Problem: x [2048, 328, 256] f32, mask [2048, 328] bool (True = dead).
Two independent masked set-norms over the set dim per sample:
  obj  = s in [0, 128)
  road = s in [128, 328)
out = (x*alive - mean) / std * w + b   with per-(sample, feature) stats:
  counts = clip(sum(alive), 1);  ok = counts > 1
  mean = sum(x*alive)/counts                   (raw sum when !ok)
  var  = sum((x*alive - mean)^2)/counts        (over ALL s, dead included)
  std  = ok ? sqrt(var + 1e-6) : 1.0
Rewrite: out[s,d] = alive_s * x[s,d] * A[d] + C[d] with A = istd*w and
C = b - mean*istd*w (dead rows get exactly (0-mean)/std*w + b).

Per-core design (256 samples, CHUNK=32):
  - layout A: s on partitions, d free. Per sample three segment tiles:
    obj [128,256], r1 [128,256], r2 [72,256].
  - stats: s1 = sum alive*x, s2 = sum alive*x^2 via TensorE (fp32r)
    accumulated across the chunk into PSUM [32, 512] at base partition 0.
    The stationary is a one-hot "mega" tile mega[s, 33*bi] = alive_bi[s]
    (else 0), built per chunk by one PE transpose-matmul per segment:
    mega = alive_segᵀ @ R with the constant R[b, 33b] = 1.
    x^2 via ScalarE Square.
  - phase2 per chunk (batch on partitions): ACo = [A | C] [32, 512] per
    segment; istd = exp(-0.5*ln(var + eps)) on ScalarE.
  - apply: per sample the PSUM broadcast [M1 | Cb] = [alive x A | 1 x C]
    comes from two matmuls with contraction over the 32 chunk rows:
      M1 = alive_sliceᵀ @ (ACo[:, :256] * onehot_bi)
      Cb = ones32ᵀ     @ (ACo[:, 256:] * onehot_bi)
    where the one-hot column masking (GpSimd tensor_scalar, eye32 col bi)
    keeps every operand at base partition 0 (matmul tile_position rule).
    Then out = x*M1 + Cb as two VectorE tensor_tensor passes; DMA out.
"""
import sys

if "/opt/trn_rl_repo" not in sys.path:
    sys.path.insert(0, "/opt/trn_rl_repo")

from contextlib import ExitStack

import numpy as np

import concourse.bacc as bacc
import concourse.bass as bass
import concourse.tile as tile
from concourse import mybir
from concourse.bass_utils import run_bass_kernel_spmd

F32 = mybir.dt.float32
F32R = mybir.dt.float32r
U8 = mybir.dt.uint8
AF = mybir.ActivationFunctionType
OP = mybir.AluOpType

NCORES = 8
B, S, D = 2048, 328, 256
B_LOC = B // NCORES  # 256
S_OBJ = 128
S_R1 = 128
S_R2 = 72
N_ROAD = S_R1 + S_R2  # 200
CHUNK = 32
EPS = 1e-6

SEGTILES = (  # (name, seg, s0, rows)
    ("obj", "obj", 0, S_OBJ),
    ("r1", "road", S_OBJ, S_R1),
    ("r2", "road", S_OBJ + S_R1, S_R2),
)

_NC_CACHE = {}


def build_nc():
    nc = bacc.Bacc("TRN2", target_bir_lowering=False, debug=False, num_devices=NCORES)
    x_d = nc.declare_dram_parameter("x", [B_LOC, S, D], F32R, isOutput=False)
    mask_d = nc.declare_dram_parameter("mask", [B_LOC, S], U8, isOutput=False)
    w_obj_d = nc.declare_dram_parameter("weights_obj", [D], F32, isOutput=False)
    b_obj_d = nc.declare_dram_parameter("biases_obj", [D], F32, isOutput=False)
    w_road_d = nc.declare_dram_parameter("weights_road", [D], F32, isOutput=False)
    b_road_d = nc.declare_dram_parameter("biases_road", [D], F32, isOutput=False)
    eye_d = nc.declare_dram_parameter("eye32", [CHUNK, CHUNK], F32, isOutput=False)
    exp_d = nc.declare_dram_parameter(
        "expand", [CHUNK, CHUNK * CHUNK], F32R, isOutput=False
    )
    out_d = nc.declare_dram_parameter("out", [B_LOC, S, D], F32, isOutput=True)

    def bcast_ap(handle, n_part, free):
        # zero-stride partition dim: DMA-broadcast a DRAM vector to n_part rows
        return bass.AP(tensor=handle, offset=0, ap=[[0, n_part], [1, free]])

    with tile.TileContext(nc) as tc, ExitStack() as ctx:
        singles = ctx.enter_context(tc.tile_pool(name="singles", bufs=1))
        chunkp = ctx.enter_context(tc.tile_pool(name="chunkp", bufs=2))
        xpool = ctx.enter_context(tc.tile_pool(name="xpool", bufs=CHUNK + 1))
        sqpool = ctx.enter_context(tc.tile_pool(name="sqpool", bufs=3))
        ohpool = ctx.enter_context(tc.tile_pool(name="ohpool", bufs=3))
        outp = ctx.enter_context(tc.tile_pool(name="outp", bufs=3))
        psum = ctx.enter_context(tc.tile_pool(name="psum", bufs=8, space="PSUM"))

        # ---- constants ----
        ones_f = singles.tile([CHUNK, 128], F32)
        nc.vector.memset(ones_f, 1.0)
        ones32 = singles.tile([CHUNK, 128], F32R)
        nc.vector.tensor_scalar(ones32, ones_f, 1.0, None, OP.mult)
        eps_t = singles.tile([CHUNK, 1], F32)
        nc.vector.memset(eps_t, EPS)
        eye32 = singles.tile([CHUNK, CHUNK], F32)
        nc.sync.dma_start(out=eye32, in_=eye_d[:, :])
        expand = singles.tile([CHUNK, CHUNK * CHUNK], F32R)
        nc.sync.dma_start(out=expand, in_=exp_d[:, :])
        wb = {}
        for nm, h in (
            ("w_obj", w_obj_d),
            ("b_obj", b_obj_d),
            ("w_road", w_road_d),
            ("b_road", b_road_d),
        ):
            t = singles.tile([CHUNK, D], F32, name=f"bc_{nm}")
            nc.sync.dma_start(out=t, in_=bcast_ap(h, CHUNK, D))
            wb[nm] = t

        n_chunks = B_LOC // CHUNK
        for c in range(n_chunks):
            b0 = c * CHUNK
            # ---- mask -> alive (+ per-segment counts via accum_out) ----
            mask_u8 = chunkp.tile([CHUNK, S], U8)
            nc.sync.dma_start(out=mask_u8, in_=mask_d[b0 : b0 + CHUNK, :])
            alive = chunkp.tile([CHUNK, S], F32R)
            cnt = {
                "obj": chunkp.tile([CHUNK, 1], F32, name="cnt_obj"),
                "road": chunkp.tile([CHUNK, 1], F32, name="cnt_road"),
            }
            nc.scalar.activation(
                alive[:, 0:S_OBJ], mask_u8[:, 0:S_OBJ], AF.Copy,
                bias=1.0, scale=-1.0, accum_out=cnt["obj"],
            )
            nc.scalar.activation(
                alive[:, S_OBJ:S], mask_u8[:, S_OBJ:S], AF.Copy,
                bias=1.0, scale=-1.0, accum_out=cnt["road"],
            )

            # ---- one-hot stats stationary: mega[s, 33*bi] = alive_bi[s] ----
            mega = {}
            for nm, _seg, s0, rows in SEGTILES:
                mg = chunkp.tile(
                    [128, CHUNK * CHUNK], F32R, name=f"mega_{nm}", bufs=1
                )
                for h in range(2):
                    f0 = h * 512
                    mg_ps = psum.tile([128, 512], F32, tag="bank", name="mg_ps")
                    nc.tensor.matmul(
                        mg_ps[0:rows, :],
                        alive[:, s0 : s0 + rows],
                        expand[:, f0 : f0 + 512],
                        start=True, stop=True,
                    )
                    nc.scalar.activation(
                        mg[0:rows, f0 : f0 + 512], mg_ps[0:rows, :], AF.Copy
                    )
                mega[nm] = mg

            # ---- count-derived per-segment scalars ----
            seg_small = {}
            for nm in ("obj", "road"):
                n_seg = S_OBJ if nm == "obj" else N_ROAD
                cc = chunkp.tile([CHUNK, 1], F32, name=f"cc_{nm}")
                nc.vector.tensor_scalar(cc, cnt[nm], 1.0, None, OP.max)
                r = chunkp.tile([CHUNK, 1], F32, name=f"r_{nm}")
                nc.vector.reciprocal(r, cc)
                okt = chunkp.tile([CHUNK, 1], F32, name=f"ok_{nm}")
                nc.vector.tensor_scalar(okt, cnt[nm], 1.0, 1.0, OP.subtract, OP.min)
                nc.vector.tensor_scalar(okt, okt, 0.0, None, OP.max)
                okm = chunkp.tile([CHUNK, 1], F32, name=f"okm_{nm}")
                nc.vector.tensor_scalar(okm, okt, -1.0, 1.0, OP.mult, OP.add)
                g = chunkp.tile([CHUNK, 1], F32, name=f"g_{nm}")
                nc.vector.tensor_scalar(g, r, float(n_seg), -2.0, OP.mult, OP.add)
                seg_small[nm] = (r, okt, okm, g)

            st = {
                ("obj", 0): psum.tile([CHUNK, D], F32, tag="bank", name="st_obj_s1"),
                ("obj", 1): psum.tile([CHUNK, D], F32, tag="bank", name="st_obj_s2"),
                ("road", 0): psum.tile([CHUNK, D], F32, tag="bank", name="st_road_s1"),
                ("road", 1): psum.tile([CHUNK, D], F32, tag="bank", name="st_road_s2"),
            }

            # ---- load + square + stats accumulation ----
            x_tiles = []
            for bi in range(CHUNK):
                b = b0 + bi
                xt = {}
                for nm, seg, s0, rows in SEGTILES:
                    xx = xpool.tile([128, D], F32R, tag=f"x_{nm}", name=f"x_{nm}")
                    nc.sync.dma_start(out=xx[0:rows, :], in_=x_d[b, s0 : s0 + rows, :])
                    ss = sqpool.tile([128, D], F32R, tag=f"sq_{nm}", name=f"sq_{nm}")
                    nc.scalar.activation(ss[0:rows, :], xx[0:rows, :], AF.Square)
                    mg = mega[nm][0:rows, CHUNK * bi : CHUNK * (bi + 1)]
                    if seg == "obj":
                        first, last = bi == 0, bi == CHUNK - 1
                    else:
                        first = bi == 0 and nm == "r1"
                        last = bi == CHUNK - 1 and nm == "r2"
                    nc.tensor.matmul(
                        st[(seg, 0)][:, :], mg, xx[0:rows, :],
                        start=first, stop=last,
                    )
                    nc.tensor.matmul(
                        st[(seg, 1)][:, :], mg, ss[0:rows, :],
                        start=first, stop=last,
                    )
                    xt[nm] = xx
                x_tiles.append(xt)

            # ---- phase2: ACo = [A | C] per segment on [CHUNK, 512] ----
            ACo = {}
            for nm in ("obj", "road"):
                r, okt, okm, g = seg_small[nm]
                s1 = st[(nm, 0)][:, :]
                s2 = st[(nm, 1)][:, :]
                mean = chunkp.tile([CHUNK, D], F32, name=f"mean_{nm}")
                nc.vector.tensor_scalar(mean, s1, r, None, OP.mult)
                var = chunkp.tile([CHUNK, D], F32, name=f"var_{nm}")
                nc.vector.tensor_mul(var, mean, mean)
                nc.vector.tensor_scalar(var, var, g, None, OP.mult)
                v1 = chunkp.tile([CHUNK, D], F32, name=f"v1_{nm}")
                nc.vector.tensor_scalar(v1, s2, r, None, OP.mult)
                nc.vector.tensor_add(var, var, v1)
                istd = chunkp.tile([CHUNK, D], F32, name=f"istd_{nm}")
                nc.scalar.activation(istd, var, AF.Ln, bias=eps_t[:, :])
                nc.scalar.activation(istd, istd, AF.Exp, scale=-0.5)
                nc.vector.tensor_scalar(istd, istd, okt, okm, OP.mult, OP.add)
                ac = chunkp.tile([CHUNK, 2 * D], F32, name=f"ACo_{nm}")
                nc.vector.tensor_mul(ac[:, 0:D], istd, wb[f"w_{nm}"])
                nc.vector.tensor_mul(ac[:, D : 2 * D], mean, ac[:, 0:D])
                nc.vector.tensor_sub(
                    ac[:, D : 2 * D], wb[f"b_{nm}"], ac[:, D : 2 * D]
                )
                ACo[nm] = ac

            # ---- apply ----
            for bi in range(CHUNK):
                b = b0 + bi
                xt = x_tiles[bi]
                oh = {}
                for seg in ("obj", "road"):
                    t = ohpool.tile(
                        [CHUNK, 2 * D], F32R, tag=f"oh_{seg}", name=f"oh_{seg}"
                    )
                    nc.gpsimd.tensor_scalar(
                        t, ACo[seg], eye32[:, bi : bi + 1], None, OP.mult
                    )
                    oh[seg] = t
                for nm, seg, s0, rows in SEGTILES:
                    mc = psum.tile([128, 2 * D], F32, tag="bank", name=f"mc_{nm}")
                    nc.tensor.matmul(
                        mc[0:rows, 0:D],
                        alive[:, s0 : s0 + rows],
                        oh[seg][:, 0:D],
                        start=True, stop=True,
                    )
                    nc.tensor.matmul(
                        mc[0:rows, D : 2 * D],
                        ones32[:, 0:rows],
                        oh[seg][:, D : 2 * D],
                        start=True, stop=True,
                    )
                    ot = outp.tile([128, D], F32, tag=f"o_{nm}", name=f"o_{nm}")
                    nc.vector.tensor_mul(
                        ot[0:rows, :], xt[nm][0:rows, :].bitcast(F32), mc[0:rows, 0:D]
                    )
                    nc.vector.tensor_add(
                        ot[0:rows, :], ot[0:rows, :], mc[0:rows, D : 2 * D]
                    )
                    nc.scalar.dma_start(
                        out=out_d[b, s0 : s0 + rows, :], in_=ot[0:rows, :]
                    )

    nc.compile()
    return nc


def _expand_const():
    # R[b, 33b] = 1: megaᵀ-expander for the one-hot stats stationary
    r = np.zeros((CHUNK, CHUNK * CHUNK), dtype=np.float32)
    for b_ in range(CHUNK):
        r[b_, (CHUNK + 1) * b_] = 1.0
    return r


def _get_nc():
    if "nc" not in _NC_CACHE:
        _NC_CACHE["nc"] = build_nc()
    return _NC_CACHE["nc"]


def kernel(x, mask, weights_obj, biases_obj, weights_road, biases_road, _trace=False):
    x = np.ascontiguousarray(np.asarray(x, dtype=np.float32))
    mask_u8 = np.ascontiguousarray(np.asarray(mask)).astype(np.uint8)
    w_obj = np.ascontiguousarray(np.asarray(weights_obj, dtype=np.float32))
    b_obj = np.ascontiguousarray(np.asarray(biases_obj, dtype=np.float32))
    w_road = np.ascontiguousarray(np.asarray(weights_road, dtype=np.float32))
    b_road = np.ascontiguousarray(np.asarray(biases_road, dtype=np.float32))

    xs = x.reshape(NCORES, B_LOC, S, D)
    ms = mask_u8.reshape(NCORES, B_LOC, S)
    eye = np.eye(CHUNK, dtype=np.float32)
    expand = _expand_const()
    in_maps = [
        {
            "x": xs[i],
            "mask": ms[i],
            "weights_obj": w_obj,
            "biases_obj": b_obj,
            "weights_road": w_road,
            "biases_road": b_road,
            "eye32": eye,
            "expand": expand,
        }
        for i in range(NCORES)
    ]
    nc = _get_nc()
    res = run_bass_kernel_spmd(nc, in_maps, core_ids=list(range(NCORES)), trace=_trace)
    out = np.concatenate([res.results[i]["out"] for i in range(NCORES)], axis=0)
    if _trace:
        kernel.last_exec_time_ns = res.exec_time_ns
        kernel.last_mean_exec_time_ns = res.mean_exec_time_ns
    return out.reshape(B, S, D)
